# revision 18
# baseline (speedup 1.0000x reference)
"""Trainium2 Bass kernel for nn_ExperimentNet (SE-style pooling net).

Reference computation (per batch b):
    pool = mean(x[b], axis=(H,W))                # (C,)
    f    = sigmoid(relu(pool @ W1.T) @ W2.T)     # (C,)
    p    = mean(x[b] * f[:,None,None], (H,W))    # (C,)  == f * pool  (f const over H,W)
    out  = p @ W3.T + b3                         # (2,)

Key algebraic identity: mean(x * f) over (H,W) equals f * mean(x), so x is
read exactly ONCE (512 MB total).  Everything after the pooling is a tiny
MLP on (B, C) = (32, 256) values.

Strategy: pure data parallel over 8 NeuronCores, 4 batches per core.
Per core: stream the (4*256, 16384) row-major shard through SBUF, reduce
over the free (spatial) dim on DVE, then run the whole MLP on-chip.

Tail minimization (the DMA stream itself is at the ~358 GB/s per-NC HBM
roofline, so the only wins left are at the edges):
  * groups are streamed c-major (all channel-chunk-0 groups first), so the
    f1 = W1 @ pool contraction over the first 128 channels runs mid-stream;
  * the whole MLP is pipelined per batch COLUMN: batch b's chain (f1 c=1
    matmul -> relu -> f2 -> sigmoid -> *pool -> out-psum) is emitted right
    after its last pool-reduce, so after the final x byte only batch 3's
    tiny chain remains;
  * the final group's last chunk shrinks geometrically (2048,1024,512,256,
    256 cols), so the last reduce covers 256 cols instead of 4096 and the
    two halves go on DVE and ACT in parallel;
  * the output is produced transposed (2, B_LOC) so the +b3 bias folds into
    the single ACT copy (per-partition bias), saving a DVE hop.

The 1/(H*W) mean scaling is folded into host-prepared W1.T and W3.T copies
(exact: 16384 is a power of two), so the kernel only ever needs raw sums.
"""

import numpy as np

import concourse.bacc as bacc
import concourse.bass as bass
import concourse.mybir as mybir
from concourse import tile
from concourse.bass_utils import run_bass_kernel_spmd

N_CORES = 8
B, C, H, W = 32, 256, 128, 128
S = H * W                  # 16384 spatial elements per (b, c)
B_LOC = B // N_CORES       # 4 batches per core
ROWS = B_LOC * C           # 1024 (b, c) rows per core
P = 128                    # SBUF partitions
G = ROWS // P              # 8 row groups per core
CR = C // 4                # 64 hidden units
KC = C // P                # 2 contraction chunks of 128 for C-dim matmuls

FP32 = mybir.dt.float32

_CACHE = {}


def _build_nc(ch=8192, bufs=4, act_frac=0.5, tail_geo=True, tail_min=512,
              reps=1, serialize_reps=True, dual_ring=False, rings=None,
              loop_reps=0, tail_par=True,
              no_mlp=False, empty=False, half_all=False):
    """Build the per-core bass program.

    ch: free-dim chunk per DMA; bufs: xin double-buffer depth;
    act_frac: fraction of chunk reductions routed to ScalarE (ACT) instead
    of VectorE (DVE); tail_geo: shrink the final group's last chunk
    geometrically down to tail_min cols so the last reduce is tiny.
    reps / serialize_reps / loop_reps: benchmarking-only repetition (see
    test.py; each For_i back-edge is a full all-engine barrier).
    """
    nch = S // ch
    nc = bacc.Bacc("TRN2", target_bir_lowering=False, debug=False)
    if rings is None:
        rings = ["sync", "scalar"] if dual_ring else ["sync"]

    x_d = nc.dram_tensor("x", [ROWS, S], FP32, kind="ExternalInput")
    w1t_d = nc.dram_tensor("w1t", [C, CR], FP32, kind="ExternalInput")   # W1.T / S
    w2t_d = nc.dram_tensor("w2t", [CR, C], FP32, kind="ExternalInput")   # W2.T
    w3t_d = nc.dram_tensor("w3t", [C, 2], FP32, kind="ExternalInput")    # W3.T / S
    b3b_d = nc.dram_tensor("b3b", [2, 1], FP32, kind="ExternalInput")
    out_d = nc.dram_tensor("out", [2, B_LOC], FP32, kind="ExternalOutput")

    with tile.TileContext(nc) as tc:
        with (
            tc.tile_pool(name="xin", bufs=bufs) as xpool,
            tc.tile_pool(name="small", bufs=1) as spool,
            tc.tile_pool(name="stage", bufs=4) as stpool,
            tc.tile_pool(name="psum", bufs=1, space="PSUM") as ppool,
        ):
            # --- persistent small tiles -------------------------------------
            # Weight loads go on the ACT HWDGE ring so they don't delay the
            # x-stream at the head of the sync ring's FIFO.
            w_eng = nc.scalar
            w1t = []
            w3t = []
            for c in range(KC):
                t1 = spool.tile([P, CR], FP32, tag=f"w1t{c}", name=f"w1t{c}")
                w_eng.dma_start(t1[:], w1t_d[c * P:(c + 1) * P, :])
                w1t.append(t1)
                t3 = spool.tile([P, 2], FP32, tag=f"w3t{c}", name=f"w3t{c}")
                w_eng.dma_start(t3[:], w3t_d[c * P:(c + 1) * P, :])
                w3t.append(t3)
            w2t = spool.tile([CR, C], FP32, tag="w2t")
            w_eng.dma_start(w2t[:], w2t_d[:])
            b3b = spool.tile([2, 1], FP32, tag="b3b")
            w_eng.dma_start(b3b[:], b3b_d[:])

            def body(rep):
                if empty:
                    # timing diagnostic: barrier-only loop body
                    z = spool.tile([2, B_LOC], FP32, tag="resT")
                    nc.vector.tensor_scalar_mul(z[:], w2t[0:2, 0:B_LOC], 0.0)
                    return
                # poolT[c][p, b] = sum over spatial of x[b, c*128+p, :, :]
                poolT = [
                    spool.tile([P, B_LOC], FP32, tag=f"poolT{c}",
                               name=f"poolT{c}_{rep}")
                    for c in range(KC)
                ]
                ps_f1 = ppool.tile([CR, B_LOC], FP32, tag="ps_f1")
                ps_f2 = [
                    ppool.tile([P, B_LOC], FP32, tag=f"ps_f2{c}",
                               name=f"ps_f2{c}_{rep}")
                    for c in range(KC)
                ]
                ps_oT = ppool.tile([2, B_LOC], FP32, tag="ps_oT")
                f1 = spool.tile([CR, B_LOC], FP32, tag="f1")
                f2 = [
                    spool.tile([P, B_LOC], FP32, tag=f"f2{c}",
                               name=f"f2{c}_{rep}")
                    for c in range(KC)
                ]
                pT = [
                    spool.tile([P, B_LOC], FP32, tag=f"pT{c}",
                               name=f"pT{c}_{rep}")
                    for c in range(KC)
                ]
                resT = spool.tile([2, B_LOC], FP32, tag="resT")

                # --- streaming reduction over x, c-major group order --------
                act_acc = 0.0
                dma_i = 0
                for c_idx in range(KC):
                    for b_idx in range(B_LOC):
                        g = b_idx * KC + c_idx   # row-group in x layout
                        last_group = (c_idx == KC - 1 and b_idx == B_LOC - 1)
                        pieces = [(j * ch, ch) for j in range(nch)]
                        if last_group and tail_geo:
                            st0, w0 = pieces.pop()
                            off, rem = st0, w0
                            wsub = w0 // 2
                            while wsub >= tail_min and rem - wsub >= tail_min:
                                pieces.append((off, wsub))
                                off += wsub
                                rem -= wsub
                                wsub //= 2
                            pieces.append((off, rem))
                        split = half_all or (last_group and tail_par)
                        n_cols = 2 * len(pieces) if split else len(pieces)
                        stage = stpool.tile([P, n_cols], FP32, tag="stage")
                        for j, (col0, width) in enumerate(pieces):
                            xt = xpool.tile([P, width], FP32, tag="xt")
                            dma_eng = getattr(nc, rings[dma_i % len(rings)])
                            dma_i += 1
                            dma_eng.dma_start(
                                xt[:],
                                x_d[g * P:(g + 1) * P, col0:col0 + width],
                            )
                            if split:
                                # Critical-path group: reduce every piece's
                                # two halves on DVE and ACT in parallel so
                                # neither engine serializes the tail.
                                half = width // 2
                                nc.vector.reduce_sum(
                                    stage[:, 2 * j:2 * j + 1], xt[:, :half],
                                    axis=mybir.AxisListType.X,
                                )
                                nc.scalar.activation(
                                    xt[:, half:], xt[:, half:],
                                    mybir.ActivationFunctionType.Copy,
                                    accum_out=stage[:, 2 * j + 1:2 * j + 2],
                                )
                                continue
                            act_acc += act_frac
                            if act_acc >= 1.0:
                                act_acc -= 1.0
                                nc.scalar.activation(
                                    xt[:], xt[:],
                                    mybir.ActivationFunctionType.Copy,
                                    accum_out=stage[:, j:j + 1],
                                )
                            else:
                                nc.vector.reduce_sum(
                                    stage[:, j:j + 1], xt[:],
                                    axis=mybir.AxisListType.X,
                                )
                        bb = b_idx
                        nc.vector.reduce_sum(
                            poolT[c_idx][:, bb:bb + 1], stage[:],
                            axis=mybir.AxisListType.X,
                        )

                        # --- per-batch-column MLP pipelining ----------------
                        if no_mlp:
                            continue
                        if c_idx == 0:
                            # first half of the f1 contraction, mid-stream
                            nc.tensor.matmul(
                                ps_f1[:, bb:bb + 1], w1t[0][:],
                                poolT[0][:, bb:bb + 1],
                                start=True, stop=False,
                            )
                        else:
                            nc.tensor.matmul(
                                ps_f1[:, bb:bb + 1], w1t[1][:],
                                poolT[1][:, bb:bb + 1],
                                start=False, stop=True,
                            )
                            nc.scalar.activation(
                                f1[:, bb:bb + 1], ps_f1[:, bb:bb + 1],
                                mybir.ActivationFunctionType.Relu,
                            )
                            for c in range(KC):
                                nc.tensor.matmul(
                                    ps_f2[c][:, bb:bb + 1],
                                    w2t[:, c * P:(c + 1) * P],
                                    f1[:, bb:bb + 1],
                                    start=True, stop=True,
                                )
                                nc.scalar.activation(
                                    f2[c][:, bb:bb + 1],
                                    ps_f2[c][:, bb:bb + 1],
                                    mybir.ActivationFunctionType.Sigmoid,
                                )
                                nc.vector.tensor_mul(
                                    pT[c][:, bb:bb + 1],
                                    f2[c][:, bb:bb + 1],
                                    poolT[c][:, bb:bb + 1],
                                )
                                nc.tensor.matmul(
                                    ps_oT[:, bb:bb + 1], w3t[c][:],
                                    pT[c][:, bb:bb + 1],
                                    start=(c == 0), stop=(c == KC - 1),
                                )
                            # resT col = ps_oT col + b3 (per-partition bias);
                            # emitted per batch so only batch 3's is in the
                            # post-stream tail
                            nc.scalar.activation(
                                resT[:, bb:bb + 1], ps_oT[:, bb:bb + 1],
                                mybir.ActivationFunctionType.Identity,
                                bias=b3b[:],
                            )

                if no_mlp:
                    # timing diagnostic: stream+reduce only, dummy output
                    nc.vector.tensor_scalar_mul(
                        resT[:], w2t[0:2, 0:B_LOC], 0.0)
                nc.scalar.dma_start(out_d[:], resT[:])

            if loop_reps:
                # Dynamic loop for benchmarking: each back-edge is a full
                # all-engine barrier (+ sem reset), so iterations serialize
                # like independent executions.  Tiny NEFF, huge device time.
                with tc.For_i(0, loop_reps, 1):
                    body(0)
            else:
                for rep in range(reps):
                    if rep > 0 and serialize_reps:
                        tc.strict_bb_all_engine_barrier()
                    body(rep)

    nc.compile()
    return nc


def _get_nc(**kw):
    key = tuple(sorted(kw.items()))
    if key not in _CACHE:
        _CACHE[key] = _build_nc(**kw)
    return _CACHE[key]


def kernel(x, W1, W2, W3, b3, **_unused):
    x = np.ascontiguousarray(np.asarray(x, dtype=np.float32))
    w1t = (np.asarray(W1, np.float32).T / np.float32(S)).astype(np.float32)
    w1t = np.ascontiguousarray(w1t)                       # (C, CR)
    w2t = np.ascontiguousarray(np.asarray(W2, np.float32).T)   # (CR, C)
    w3t = np.ascontiguousarray(
        (np.asarray(W3, np.float32).T / np.float32(S)).astype(np.float32)
    )                                                     # (C, 2)
    b3b = np.ascontiguousarray(
        np.asarray(b3, np.float32).reshape(2, 1)
    )

    nc = _get_nc()
    in_maps = [
        {
            "x": x[i * B_LOC:(i + 1) * B_LOC].reshape(ROWS, S),
            "w1t": w1t,
            "w2t": w2t,
            "w3t": w3t,
            "b3b": b3b,
        }
        for i in range(N_CORES)
    ]
    res = run_bass_kernel_spmd(nc, in_maps, list(range(N_CORES)))
    # per-core out is transposed (2, B_LOC); batch b = core*B_LOC + col
    out = np.concatenate(
        [res.results[i]["out"].T for i in range(N_CORES)], axis=0
    )
    return out.astype(np.float32)
